# revision 47
# baseline (speedup 1.0000x reference)
"""Trainium2 Bass kernel: causal multi-head self-attention with RoPE.

Problem: x[2,2048,1024], 16 heads, d_k=64, causal, RoPE(theta=1e4),
out = (softmax(rope(Q)rope(K)^T/8) V) WO^T.

Sharding (8 cores): data-parallel over batch (2) x head-parallel over
head groups (4 heads per core).  Each core computes Q/K/V projections
for its 4 heads, flash-style causal attention, and a partial output
projection over its 256 channels; the host sums the 4 partials per
batch element.

v4 design (head-pair-major, projection fillers):
  - All input DMAs ride one prioritized HW-DGE queue in consumption
    order (wq, x0, wk, x1, ...), so the first projection chain starts
    as soon as ~1.5MB has landed.
  - Phase A: Q/K projections for head-pair 0 (ot=0) + RoPE + V blocks
    0-3 only.
  - Phase B: attention for head-pair 0 across all 4 query slices.  The
    Act engine is the pacer (exp); the PE's idle slots are filled with
    the remaining projection work (V blocks 4-15, Q/K ot=1 chains,
    RoPE ot=1) drained from a deadline-ordered filler queue.
  - Phase C: attention for head-pair 1, with softmax normalization +
    output projection + stores rolling through the same filler
    mechanism.
  - exp skips fully-masked leading columns; the causal triangle of
    diagonal 128x128 blocks is zeroed multiplicatively post-exp on DVE.
  - psum->sbuf copies: Act during phase A (idle), DVE in B/C.
"""

import os
import sys
from collections import deque

for _p in ("/opt/trn_rl_repo",):
    if _p not in sys.path:
        sys.path.insert(0, _p)

import numpy as np
import ml_dtypes

BF16 = ml_dtypes.bfloat16

D = 1024
S = 2048
H = 16
DK = 64
HPC = 4          # heads per core
NCORES = 8
THETA = 10000.0

_COMPILED = {}


def _build_nc():
    import concourse.bass as bass  # noqa: F401
    import concourse.bacc as bacc
    import concourse.mybir as mybir
    import concourse.tile as tile

    bf16 = mybir.dt.bfloat16
    f32 = mybir.dt.float32
    Exp = mybir.ActivationFunctionType.Exp

    nc = bacc.Bacc(
        "TRN2", target_bir_lowering=False, debug=False, num_devices=NCORES
    )
    xt0_d = nc.declare_dram_parameter("xt0", [2, 128, 8, 256], bf16, isOutput=False)
    xt_d = nc.declare_dram_parameter("xt", [3, 128, 8, 512], bf16, isOutput=False)
    wq_d = nc.declare_dram_parameter("wq", [128, 8, 256], bf16, isOutput=False)
    wk_d = nc.declare_dram_parameter("wk", [128, 8, 256], bf16, isOutput=False)
    wv_d = nc.declare_dram_parameter("wv", [128, 8, 256], bf16, isOutput=False)
    wo_d = nc.declare_dram_parameter("wo", [128, 2, D], bf16, isOutput=False)
    cos_d = nc.declare_dram_parameter("cosb", [128, S], bf16, isOutput=False)
    sin_d = nc.declare_dram_parameter("sinb", [128, S], bf16, isOutput=False)
    tri_d = nc.declare_dram_parameter("tri", [128, 128], bf16, isOutput=False)
    prm_d = nc.declare_dram_parameter("prm", [128, 128], bf16, isOutput=False)
    ind_d = nc.declare_dram_parameter("ind2", [2, 128], bf16, isOutput=False)
    out_d = nc.declare_dram_parameter("out", [S, D], bf16, isOutput=True)

    with tile.TileContext(nc) as tc:
        with tc.tile_pool(name="const", bufs=1) as const:
            # x slice 0 split in halves so the first chain starts sooner
            x0h = [const.tile([128, 8, 256], bf16, name=f"x0{i}") for i in range(2)]
            x_sb = [None] + [const.tile([128, 8, 512], bf16, name=f"x{i}")
                             for i in range(1, 4)]
            scr_w = const.tile([128, 128], bf16)
            scr_x = const.tile([128, 512], bf16)
            wq_sb = const.tile([128, 8, 256], bf16)
            wk_sb = const.tile([128, 8, 256], bf16)
            wv_sb = const.tile([128, 8, 256], bf16)
            wo_sb = const.tile([128, 2, D], bf16)
            cos_sb = const.tile([128, S], bf16)
            sin_sb = const.tile([128, S], bf16)
            tri_sb = const.tile([128, 128], bf16)
            prm_sb = const.tile([128, 128], bf16)
            ind_sb = const.tile([2, 128], bf16)
            v_sb = const.tile([128, 16, 4, 65], bf16)
            qraw = [const.tile([128, S], bf16, name=f"qraw{i}") for i in range(2)]
            kraw = [const.tile([128, S], bf16, name=f"kraw{i}") for i in range(2)]
            # rotated Q/K in per-half tiles so attention starts as soon as
            # the first half is roped
            qrot = [[const.tile([128, 1024], bf16, name=f"qrot{i}{hf}")
                     for hf in range(2)] for i in range(2)]
            krot = [[const.tile([128, 1024], bf16, name=f"krot{i}{hf}")
                     for hf in range(2)] for i in range(2)]
            # unnormalized / normalized head outputs, per (ot, jsl)
            atj = [[const.tile([128, 512], bf16, name=f"at{o}_{j}")
                    for j in range(4)] for o in range(2)]
            atn = [[const.tile([128, 512], bf16, name=f"an{o}_{j}")
                    for j in range(4)] for o in range(2)]
            den_sb = const.tile([2, 8, 512], bf16)  # [hl, jsl*2+ot, q]

            # All inputs in-order on the single HW-DGE (sync) queue so the
            # full HBM bandwidth goes to each in consumption order.
            nc.sync.dma_start(wq_sb[:], wq_d[:])
            nc.sync.dma_start(x0h[0][:], xt0_d[0])
            nc.sync.dma_start(wk_sb[:], wk_d[:])
            nc.sync.dma_start(x0h[1][:], xt0_d[1])
            nc.sync.dma_start(prm_sb[:], prm_d[:])
            nc.sync.dma_start(x_sb[1][:], xt_d[0])
            nc.sync.dma_start(cos_sb[:], cos_d[:])
            nc.sync.dma_start(sin_sb[:], sin_d[:])
            nc.sync.dma_start(x_sb[2][:], xt_d[1])
            nc.sync.dma_start(x_sb[3][:], xt_d[2])
            nc.sync.dma_start(wv_sb[:], wv_d[:])
            nc.sync.dma_start(tri_sb[:], tri_d[:])
            nc.sync.dma_start(ind_sb[:], ind_d[:])
            nc.sync.dma_start(wo_sb[:], wo_d[:])
            nc.vector.memset(v_sb[:, :, :, 64:65], 1.0)
            nc.vector.memset(scr_w[:], 0.0)
            nc.vector.memset(scr_x[:], 0.0)

            with tc.tile_pool(name="rope", bufs=1) as rp:

                def rope_half(raw, rot, ot, half, pool, ptag):
                    # partition swap via permutation matmul on the PE: no
                    # DMA-queue latency on the critical path
                    base = half * 1024
                    swps = []
                    for ch in range(2):
                        swp = pool.tile([128, 512], f32, tag=ptag,
                                        name="swp", bufs=2)
                        nc.tensor.matmul(
                            swp[:], prm_sb[:],
                            raw[ot][:, base + ch * 512:base + (ch + 1) * 512],
                            start=True, stop=True,
                        )
                        swps.append(swp)
                    t1 = rp.tile([128, 1024], bf16, tag="t1", name="t1",
                                 bufs=2)
                    s2 = rp.tile([128, 1024], bf16, tag="s2", name="s2",
                                 bufs=2)
                    nc.vector.tensor_mul(
                        t1[:], raw[ot][:, base:base + 1024],
                        cos_sb[:, base:base + 1024])
                    for ch in range(2):
                        nc.vector.tensor_mul(
                            s2[:, ch * 512:(ch + 1) * 512], swps[ch][:],
                            sin_sb[:, base + ch * 512:base + (ch + 1) * 512])
                    nc.vector.tensor_add(rot[ot][half][:], t1[:], s2[:])

                # (tile, column offset in S-raw layout, width)
                x_parts = [(x0h[0], 0, 256), (x0h[1], 256, 256),
                           (x_sb[1], 512, 512), (x_sb[2], 1024, 512),
                           (x_sb[3], 1536, 512)]

                def qk_chain(ot, part, w_sb, raw, ps, act_copy):
                    xt_, col0, width = x_parts[part]
                    for c in range(8):
                        nc.tensor.matmul(
                            ps[:, 0:width],
                            w_sb[:, c, ot * 128:(ot + 1) * 128],
                            xt_[:, c, :],
                            start=(c == 0), stop=(c == 7),
                        )
                    dst = raw[ot][:, col0:col0 + width]
                    if act_copy:
                        nc.scalar.copy(dst, ps[:, 0:width])
                    else:
                        nc.vector.tensor_copy(dst, ps[:, 0:width])

                def v_chain(sb, ps, act_copy):
                    if sb < 4:
                        xt_ = x0h[sb // 2]
                        lo = (sb % 2) * 128
                    else:
                        xt_ = x_sb[sb // 4]
                        lo = (sb % 4) * 128
                    for c in range(8):
                        nc.tensor.matmul(
                            ps[:, 0:256],
                            xt_[:, c, lo:lo + 128],
                            wv_sb[:, c, :],
                            start=(c == 0), stop=(c == 7),
                        )
                    src = ps[:, 0:256].rearrange("p (h d) -> p h d", h=4)
                    if act_copy:
                        nc.scalar.copy(v_sb[:, sb, :, 0:64], src)
                    else:
                        nc.vector.tensor_copy(v_sb[:, sb, :, 0:64], src)

                # ---- phase A ----
                with tc.tile_pool(name="pj", bufs=1, space="PSUM") as pjp:
                    # warm up the PE clock on scratch data while the first
                    # input DMAs land
                    wps = pjp.tile([128, 512], f32, tag="warm", name="wps",
                                   bufs=1)
                    for _ in range(12):
                        nc.tensor.matmul(wps[:], scr_w[:], scr_x[:],
                                         start=True, stop=True)
                    for part in range(5):
                        for w_sb, raw in ((wq_sb, qraw), (wk_sb, kraw)):
                            ps = pjp.tile([128, 512], f32, tag="pj",
                                          name="pj", bufs=4)
                            qk_chain(0, part, w_sb, raw, ps, act_copy=True)
                        if part == 2:
                            rope_half(qraw, qrot, 0, 0, pjp, "sw")
                            rope_half(kraw, krot, 0, 0, pjp, "sw")
                    rope_half(qraw, qrot, 0, 1, pjp, "sw")
                    rope_half(kraw, krot, 0, 1, pjp, "sw")
                    for sb in range(4):
                        ps = pjp.tile([128, 512], f32, tag="pj", name="pv",
                                      bufs=4)
                        v_chain(sb, ps, act_copy=True)

                # ---- phases B/C ----
                with tc.tile_pool(name="ps_s", bufs=1, space="PSUM") as psc, \
                     tc.tile_pool(name="ps_o", bufs=1, space="PSUM") as pso, \
                     tc.tile_pool(name="ps_f", bufs=1, space="PSUM") as psf, \
                     tc.tile_pool(name="pp", bufs=1) as ppool, \
                     tc.tile_pool(name="nrm", bufs=1) as nrm:

                    def make_unit(jsl, h, g0, nkb):
                        ot, hl = divmod(h, 2)
                        r0 = hl * 64
                        qr = qrot[ot][jsl // 2]
                        q0 = (jsl % 2) * 512
                        state = {}

                        def emit_scores():
                            sp = psc.tile([128, 1024], f32, tag="sc",
                                          name="sp", bufs=2)
                            pt = ppool.tile([128, 1024], bf16, tag="pt",
                                            name="pt", bufs=3)
                            state["pt"] = pt
                            dgs = []
                            for i in range(2):
                                kb = g0 + i
                                dg = kb - 4 * jsl
                                c0 = dg * 128 if dg > 0 else 0
                                kr = krot[ot][kb // 8]
                                k0 = (kb % 8) * 128
                                nc.tensor.matmul(
                                    sp[:, i * 512 + c0:(i + 1) * 512],
                                    kr[r0:r0 + 64, k0:k0 + 128],
                                    qr[r0:r0 + 64, q0 + c0:q0 + 512],
                                    start=True, stop=True,
                                )
                                dgs.append(dg)
                            # exp: skip fully-masked leading columns of the
                            # deep diagonal blocks
                            if dgs[0] >= 2:
                                for i in range(2):
                                    c0 = dgs[i] * 128
                                    nc.scalar.activation(
                                        pt[:, i * 512 + c0:(i + 1) * 512],
                                        sp[:, i * 512 + c0:(i + 1) * 512],
                                        Exp, scale=0.125,
                                    )
                            else:
                                nc.scalar.activation(
                                    pt[:, 0:1024], sp[:, 0:1024],
                                    Exp, scale=0.125,
                                )
                            # zero the causal triangle of diagonal blocks
                            for i in range(2):
                                dg = g0 + i - 4 * jsl
                                if 0 <= dg <= 3:
                                    a = i * 512 + dg * 128
                                    nc.vector.tensor_mul(
                                        pt[:, a:a + 128], pt[:, a:a + 128],
                                        tri_sb[:],
                                    )

                        def emit_pv(po):
                            pt = state["pt"]
                            for i in range(2):
                                kb = g0 + i
                                dg = kb - 4 * jsl
                                c0 = dg * 128 if dg > 0 else 0
                                nc.tensor.matmul(
                                    po[:, c0:512],
                                    v_sb[:, kb, h, 0:65],
                                    pt[:, i * 512 + c0:(i + 1) * 512],
                                    start=(kb == 0), stop=(kb == nkb - 1),
                                )

                        return emit_scores, emit_pv

                    def emit_stage(jsl, h, po):
                        ot, hl = divmod(h, 2)
                        r0 = hl * 64
                        tm = nrm.tile([65, 512], bf16, tag="tm", name="tm",
                                      bufs=3)
                        nc.vector.tensor_copy(tm[:], po[:])
                        nc.sync.dma_start(
                            atj[ot][jsl][r0:r0 + 64, :], tm[0:64, :])
                        nc.sync.dma_start(
                            den_sb[hl:hl + 1, jsl * 2 + ot, :], tm[64:65, :])

                    def t_norm(jsl, ot):
                        denf = nrm.tile([2, 512], f32, tag="denf",
                                        name="denf", bufs=2)
                        rc2 = nrm.tile([2, 512], f32, tag="rc2", name="rc2",
                                       bufs=2)
                        rcb = nrm.tile([2, 512], bf16, tag="rcb", name="rcb",
                                       bufs=2)
                        nc.vector.tensor_copy(
                            denf[:], den_sb[0:2, jsl * 2 + ot, :])
                        nc.vector.reciprocal_approx_fast(rc2[:], denf[:])
                        nc.vector.tensor_copy(rcb[:], rc2[:])
                        rbp = psf.tile([128, 512], f32, tag="pf", name="rb",
                                       bufs=2)
                        nc.tensor.matmul(
                            rbp[:], ind_sb[0:2, :], rcb[0:2, :],
                            start=True, stop=True,
                        )
                        nc.vector.tensor_mul(
                            atn[ot][jsl][:], atj[ot][jsl][:], rbp[:])

                    def t_proj(jsl, sbi, osl):
                        pf = psf.tile([128, 512], f32, tag="pf", name="pf",
                                      bufs=2)
                        for ich in range(2):
                            nc.tensor.matmul(
                                pf[:],
                                atn[ich][jsl][:, sbi * 128:(sbi + 1) * 128],
                                wo_sb[:, ich, osl * 512:(osl + 1) * 512],
                                start=(ich == 0), stop=(ich == 1),
                            )
                        ob = nrm.tile([128, 512], bf16, tag="ob", name="ob",
                                      bufs=3)
                        nc.vector.tensor_copy(ob[:], pf[:])
                        sb = jsl * 4 + sbi
                        # final slice's stores split across two HW-DGE queues
                        # (Act is idle by then) to shorten the drain
                        eng = nc.scalar if (jsl == 3 and osl == 1) else nc.sync
                        eng.dma_start(
                            out_d[sb * 128:(sb + 1) * 128,
                                  osl * 512:(osl + 1) * 512],
                            ob[:],
                        )

                    def f_vchain(sb):
                        ps = psf.tile([128, 512], f32, tag="pf", name="pv",
                                      bufs=2)
                        v_chain(sb, ps, act_copy=False)

                    def f_qkchain(part, which):
                        w_sb, raw = ((wq_sb, qraw), (wk_sb, kraw))[which]
                        ps = psf.tile([128, 512], f32, tag="pf", name="pj",
                                      bufs=2)
                        qk_chain(1, part, w_sb, raw, ps, act_copy=False)

                    # deadline-ordered filler queue for phase B, then the
                    # rolling normalization/projection tail for phase C
                    fillers = deque()
                    for sb in range(4, 8):
                        fillers.append(lambda s=sb: f_vchain(s))
                    for part in range(3):
                        for w in range(2):
                            fillers.append(
                                lambda p=part, w_=w: f_qkchain(p, w_))
                    fillers.append(
                        lambda: rope_half(qraw, qrot, 1, 0, psf, "pf"))
                    fillers.append(
                        lambda: rope_half(kraw, krot, 1, 0, psf, "pf"))
                    for sb in range(8, 12):
                        fillers.append(lambda s=sb: f_vchain(s))
                    for part in range(3, 5):
                        for w in range(2):
                            fillers.append(
                                lambda p=part, w_=w: f_qkchain(p, w_))
                    fillers.append(
                        lambda: rope_half(qraw, qrot, 1, 1, psf, "pf"))
                    fillers.append(
                        lambda: rope_half(kraw, krot, 1, 1, psf, "pf"))
                    for sb in range(12, 16):
                        fillers.append(lambda s=sb: f_vchain(s))

                    fill_credit = [0.0]

                    def run_stream(pair, rate):
                        """Emit the attention stream for one head pair across
                        all 4 query slices, draining `fillers` at `rate` ops
                        per unit into the PE's Act-bound idle slots."""
                        hA, hB = 2 * pair, 2 * pair + 1
                        prev = None
                        for jsl in range(4):
                            nkb = 4 * (jsl + 1)
                            po = {
                                hA: pso.tile([65, 512], f32, tag="po",
                                             name=f"poA{pair}{jsl}", bufs=2),
                                hB: pso.tile([65, 512], f32, tag="po",
                                             name=f"poB{pair}{jsl}", bufs=2),
                            }
                            units = []
                            for g0 in range(0, nkb, 2):
                                units.append(
                                    (jsl, hA, g0, po, *make_unit(jsl, hA, g0, nkb)))
                                units.append(
                                    (jsl, hB, g0, po, *make_unit(jsl, hB, g0, nkb)))
                            for u in units:
                                u[4]()  # emit_scores
                                fill_credit[0] += rate
                                while fillers and fill_credit[0] >= 1.0:
                                    fill_credit[0] -= 1.0
                                    fillers.popleft()()
                                if prev is not None:
                                    pjsl, ph, pg0, ppo = prev[0], prev[1], prev[2], prev[3]
                                    prev[5](ppo[ph])  # emit_pv
                                    if pg0 + 2 >= 4 * (pjsl + 1):
                                        emit_stage(pjsl, ph, ppo[ph])
                                        if ph % 2 == 1:
                                            fillers.append(
                                                lambda j=pjsl, o=ph // 2:
                                                t_norm(j, o))
                                            if ph >= 2:
                                                for sbi in range(4):
                                                    for osl in range(2):
                                                        fillers.append(
                                                            lambda j=pjsl,
                                                            s=sbi, o=osl:
                                                            t_proj(j, s, o))
                                prev = u
                        return prev

                    prev = run_stream(0, rate=0.67)
                    # flush the last pair-0 unit before pair 1 begins
                    pjsl, ph, pg0, ppo = prev[0], prev[1], prev[2], prev[3]
                    prev[5](ppo[ph])
                    emit_stage(pjsl, ph, ppo[ph])
                    fillers.append(lambda j=pjsl: t_norm(j, 0))

                    prev = run_stream(1, rate=1.0)
                    pjsl, ph, pg0, ppo = prev[0], prev[1], prev[2], prev[3]
                    prev[5](ppo[ph])
                    emit_stage(pjsl, ph, ppo[ph])
                    fillers.append(lambda j=pjsl: t_norm(j, 1))
                    for sbi in range(4):
                        for osl in range(2):
                            fillers.append(
                                lambda j=pjsl, s=sbi, o=osl: t_proj(j, s, o))
                    while fillers:
                        fillers.popleft()()
    nc.compile()
    return nc


def _host_prep(x, token_positions, WQ, WK, WV, WO):
    """Build the 8 per-core input maps."""
    pos = np.asarray(token_positions).astype(np.float32)
    k = np.arange(DK // 2, dtype=np.float32)
    inv_freq = 1.0 / (THETA ** (2.0 * k / DK))
    ang = pos[:, None] * inv_freq[None, :]          # [S, 32]
    c32 = np.cos(ang).T.astype(np.float32)          # [32, S]
    s32 = np.sin(ang).T.astype(np.float32)
    cosb = np.tile(c32, (4, 1)).astype(BF16)        # [128, S]
    sinb = np.concatenate([-s32, s32, -s32, s32], axis=0).astype(BF16)
    # 0/1 lower-triangle for zeroing the causal triangle of diagonal blocks
    kk = np.arange(128)[:, None]
    qq = np.arange(128)[None, :]
    tri = (qq >= kk).astype(np.float32).astype(BF16)        # [128, 128]
    # permutation matrix for the RoPE partition swap (i <-> i^32)
    prm = np.zeros((128, 128), dtype=np.float32)
    prm[np.arange(128) ^ 32, np.arange(128)] = 1.0
    prm = prm.astype(BF16)
    # denominator-broadcast indicator: ind2[hl, r] = 1 iff r//64 == hl
    ind2 = np.zeros((2, 128), dtype=np.float32)
    ind2[0, 0:64] = 1.0
    ind2[1, 64:128] = 1.0
    ind2 = ind2.astype(BF16)

    perm = np.concatenate([np.arange(0, DK, 2), np.arange(1, DK, 2)])  # evens,odds

    in_maps = []
    for core in range(NCORES):
        b, hg = divmod(core, 4)
        ch0 = hg * 256
        qk_rows = np.concatenate([ch0 + hl * 64 + perm for hl in range(HPC)])
        def dev_w(w):  # [D, M] -> [128, 8, M] (contraction chunks)
            return np.ascontiguousarray(
                w.reshape(8, 128, -1).transpose(1, 0, 2)
            ).astype(BF16)

        xt = np.asarray(x[b]).T                       # [D, S]
        xt4 = np.ascontiguousarray(
            xt[:, 512:].reshape(8, 128, 3, 512).transpose(2, 1, 0, 3)
        ).astype(BF16)                                # [3, 128, 8, 512]
        xt0 = np.ascontiguousarray(
            xt[:, 0:512].reshape(8, 128, 2, 256).transpose(2, 1, 0, 3)
        ).astype(BF16)                                # [2, 128, 8, 256]
        in_maps.append({
            "xt": xt4,
            "xt0": xt0,
            "wq": dev_w(np.asarray(WQ)[qk_rows, :].T),
            "wk": dev_w(np.asarray(WK)[qk_rows, :].T),
            "wv": dev_w(np.asarray(WV)[ch0:ch0 + 256, :].T),
            "wo": np.ascontiguousarray(
                np.asarray(WO)[:, ch0:ch0 + 256].T.reshape(2, 128, D)
                .transpose(1, 0, 2)
            ).astype(BF16),
            "cosb": cosb,
            "sinb": sinb,
            "tri": tri,
            "prm": prm,
            "ind2": ind2,
        })
    return in_maps


LAST_EXEC_NS = None
LAST_RES = None


def kernel(x, token_positions, WQ, WK, WV, WO):
    global LAST_EXEC_NS, LAST_RES
    from concourse.bass_utils import run_bass_kernel_spmd

    if "nc" not in _COMPILED:
        _COMPILED["nc"] = _build_nc()
    nc = _COMPILED["nc"]

    in_maps = _host_prep(x, token_positions, WQ, WK, WV, WO)
    res = run_bass_kernel_spmd(nc, in_maps, list(range(NCORES)))
    LAST_RES = res
    LAST_EXEC_NS = res.exec_time_ns

    out = np.zeros((2, S, D), dtype=np.float32)
    for core in range(NCORES):
        out[core // 4] += np.asarray(res.results[core]["out"], dtype=np.float32)
    return out
